# revision 20
# baseline (speedup 1.0000x reference)
"""Trainium2 Bass kernel for nn_Attention_38697655337033 (sparse_attention).

GPT-OSS-style sliding-window attention block: QKV proj + YaRN RoPE + GQA
(64 Q heads / 8 KV heads, D=64, window 128, causal) + attention sinks +
o_proj.  Sharded over 8 NeuronCores tensor-parallel by head: core c owns
query heads 8c..8c+7 and KV head c.  o_proj is column-parallel over the
2880 output features (360 per core) after an AllGather of the per-core
attention outputs, chunked by 256-query window so collectives overlap
the next window's compute.

This version is a fully pipelined single-scope schedule:
  - all DRAM tensors are host-swizzled so every DMA is contiguous per
    partition (few, large DMAs; ~350 GB/s per transfer),
  - per-window pipeline: proj(w) -> rope(w) -> attn(w) -> renorm/AG(w),
    with renorm(w-1) broadcast matmuls and oproj(w-1) interleaved so the
    PE never waits on collectives until the tail,
  - rope rotate-half runs on the PE via permutation matmuls (fp32r),
  - scores A/B head-halves run concurrently in disjoint PE row-quadrants,
  - DMA issue is spread across the Sync, Activation and GpSimd queues.

Numerics: projections, scores/PV and o_proj run bf16 operands with fp32
PSUM accumulation; rope runs fp32(r); softmax has no max-subtraction and
folds the sink into the denominator (fp32).

Self-contained: hardcodes all shapes; builds and caches the Bass program
on first call.
"""

import math
import os
import sys
import types

import numpy as np

try:
    import concourse.bass as bass  # noqa: F401
except ImportError:  # pragma: no cover
    sys.path.insert(0, "/opt/trn_rl_repo")

import ml_dtypes

import concourse.bass as bass
import concourse.mybir as mybir
import concourse.tile as tile
from concourse.bass_utils import run_bass_kernel_spmd
from concourse.masks import make_identity
from concourse.tile import ScopedClock

# ---------------------------------------------------------------- constants
B, S, E = 1, 1024, 2880
H, KV, D = 64, 8, 64
WIN = 128
BASE, SCALE, ORIG = 150000.0, 32.0, 4096
BFAST, BSLOW = 32.0, 1.0
SCALING = D ** -0.5  # 0.125, exact power of two -> folded into Wq on host

N_CORES = 8
HL = H // N_CORES          # 8 local query heads
HD_L = HL * D              # 512 local q dims
EC = E // N_CORES          # 360 output columns per core
EP = 2944                  # E padded to 23*128
KT = EP // 128             # 23 contraction tiles for projections
NQT = HL // 2              # 4 head-pair tiles
NQW = S // 256             # 4 query windows of 256
KO = H * D // 128          # 32 o_proj contraction tiles
XW = KT * 256              # 5888 x columns per window

FP32 = mybir.dt.float32
FP32R = mybir.dt.float32r
BF16 = mybir.dt.bfloat16

# ------------------------------------------------------- walrus compat patch
# This container's walrus build rejects instructions with >1 sync-wait
# ("Too many sync wait commands").  Split extra waits onto same-engine NoOp
# carriers, and split the final Tile drain into one drain per wait.
_compat_done = [False]
_carrier_n = [0]


def _install_tile_compat():
    if _compat_done[0]:
        return
    _compat_done[0] = True

    orig_cal = tile.TileContext._commit_and_lower

    def patched_cal(self, inst, original_block, old_bb_map, bb_to_exit_bb):
        if isinstance(inst, mybir.Instruction):
            si = getattr(inst, "sync_info", None)
            if si is not None and len(si.on_wait) > 1:
                waits = list(si.on_wait)
                for w in waits[:-1]:
                    _carrier_n[0] += 1
                    nop = mybir.InstNoOp(
                        name=f"swsplit-{_carrier_n[0]}",
                        engine=inst.engine,
                        sync_info=mybir.SyncInfo(on_wait=[w], on_update=[]),
                        bass_nofuse=True,
                    )
                    self._commit_instruction(nop)
                inst.sync_info = mybir.SyncInfo(
                    on_wait=[waits[-1]], on_update=list(si.on_update)
                )
        return orig_cal(self, inst, original_block, old_bb_map, bb_to_exit_bb)

    tile.TileContext._commit_and_lower = patched_cal

    def patched_dab(self, tick_clock, wait_clock):
        drain_inst = self.nc.sync.drain()
        wait_clock.add_sem_waits(
            drain_inst.ins, ScopedClock({None: tick_clock.global_clock})
        )
        si = drain_inst.ins.sync_info
        if si is not None and len(si.on_wait) > 1:
            waits = list(si.on_wait)
            drain_inst.ins.sync_info = mybir.SyncInfo(on_wait=waits[:1], on_update=[])
            for i in range(1, len(waits)):
                extra = self.nc.sync.drain()
                extra.ins.sync_info = mybir.SyncInfo(
                    on_wait=waits[i : i + 1], on_update=[]
                )
        self.nc.all_engine_barrier()
        assert self.sems is not None
        popped = self.nc._tile_sem_poison_stack.pop()
        assert popped is self._sem_poison
        self.nc.clear_and_free_semaphores(list(self.sems.allocated().values()))
        self.nc.all_engine_barrier()

    tile.TileContext._drain_and_barrier = patched_dab


def _install_prof_shim():
    """antenv.axon_hooks is missing in this container; provide it so
    BASS_TRACE-style profiling paths don't crash."""
    try:
        import antenv.axon_hooks  # noqa: F401
        return
    except ImportError:
        pass
    try:
        import antenv
        from trn_agent_boot.trn_boot import _ntff_profile_via_ctypes

        hook = _ntff_profile_via_ctypes("/opt/axon/libaxon_pjrt.so")
    except Exception:
        hook = None
        try:
            import antenv
        except ImportError:
            return
    mod = types.ModuleType("antenv.axon_hooks")
    mod._hook = hook
    mod.get_axon_ntff_profile_hook = lambda: mod._hook

    def _set(h):
        mod._hook = h

    mod.set_axon_ntff_profile_hook = _set
    sys.modules["antenv.axon_hooks"] = mod
    antenv.axon_hooks = mod


# ---------------------------------------------------------------- host prep
def _rope_tables_np(positions):
    """cos/sin YaRN tables, mirroring the reference, in float32."""
    def find_dim(rot):
        return D * math.log(ORIG / (rot * 2 * math.pi)) / (2 * math.log(BASE))

    low = max(find_dim(BFAST), 0.0)
    high = min(find_dim(BSLOW), D // 2 - 1)
    if low == high:
        high += 0.001
    pos_freqs = (BASE ** (np.arange(0, D, 2, dtype=np.float32) / np.float32(D))).astype(
        np.float32
    )
    ramp = np.clip(
        (np.arange(D // 2, dtype=np.float32) - np.float32(low))
        / np.float32(high - low),
        0.0,
        1.0,
    ).astype(np.float32)
    inv_freq = (
        (np.float32(1.0) / (np.float32(SCALE) * pos_freqs)) * ramp
        + (np.float32(1.0) / pos_freqs) * (np.float32(1.0) - ramp)
    ).astype(np.float32)
    mscale = np.float32(0.1 * math.log(SCALE) + 1.0)
    ang = positions.astype(np.float32)[:, None] * inv_freq[None, :]  # [S, 32]
    emb = np.concatenate([ang, ang], axis=-1)  # [S, D]
    return (np.cos(emb) * mscale).astype(np.float32), (np.sin(emb) * mscale).astype(
        np.float32
    )


def _make_masks():
    """Multiplicative [128, 256] masks per slot in the transposed-score
    layout.  Slot s (of 3) covers key block kb = 2Q-1+s for query window Q
    (256 wide).  Entry [j, i2] is 1 when query i2 may attend key j of that
    block:
      slot0: i2 <  j           (keys one block behind the window)
      slot1: j <= i2 <= j+127  (keys in the window's first block)
      slot2: i2 >= j+128       (keys in the window's second block)
    Window 0 simply skips slot 0 (its key block doesn't exist)."""
    j = np.arange(128)[:, None]
    i2 = np.arange(256)[None, :]
    m0 = (i2 < j).astype(np.float32)
    m1 = ((i2 >= j) & (i2 <= j + 127)).astype(np.float32)
    m2 = (i2 >= j + 128).astype(np.float32)
    return m0, m1, m2


def _swap64(m):
    return (m // 64) * 64 + (m % 64 + 32) % 64


def host_prepare(hidden_states, positions, Wq, bq, Wk, bk, Wv, bv, Wo, bo, sinks):
    """Build the 8 per-core input maps (all partition-contiguous layouts)."""
    bf = ml_dtypes.bfloat16
    x = np.asarray(hidden_states, np.float32).reshape(S, E)
    xT = np.zeros((EP, S), np.float32)
    xT[:E] = x.T
    # [p, w, k, s2] flattened to [128, NQW*KT*256]
    x_sw = np.ascontiguousarray(
        xT.reshape(KT, 128, NQW, 256).transpose(1, 2, 0, 3).reshape(128, NQW * XW)
    ).astype(bf)

    cos, sin = _rope_tables_np(np.asarray(positions))
    cosT = np.ascontiguousarray(cos.T)  # [64, S]
    sinT = np.ascontiguousarray(sin.T)
    sgn = np.where(np.arange(D) < D // 2, np.float32(-1.0), np.float32(1.0))
    sinTs = sinT * sgn[:, None]
    cos2 = np.ascontiguousarray(np.concatenate([cosT, cosT], axis=0))  # [128, S]
    sin2s = np.ascontiguousarray(np.concatenate([sinTs, sinTs], axis=0))

    m0, m1, m2 = _make_masks()
    maskAB = np.ascontiguousarray(
        np.concatenate([np.concatenate([m, m], axis=1) for m in (m0, m1, m2)], axis=1)
    ).astype(bf)  # [128, 1536]

    # permutation matrices for rope rotate-half / k duplication (fp32r)
    mm_ = np.arange(128)
    prot = np.zeros((128, 128), np.float32)
    prot[_swap64(mm_), mm_] = 1.0
    pkd = np.zeros((64, 128), np.float32)
    pkd[mm_ % 64, mm_] = 1.0
    pkr = np.zeros((64, 128), np.float32)
    pkr[(mm_ % 64 + 32) % 64, mm_] = 1.0

    # renorm broadcast row-selectors: sel[r, t*128+p] = 1 iff r == 2t + p//64
    sel = np.zeros((8, NQT, 128), np.float32)
    for t in range(NQT):
        sel[2 * t, t, 0:64] = 1.0
        sel[2 * t + 1, t, 64:128] = 1.0
    sel = np.ascontiguousarray(sel.reshape(8, NQT * 128))

    Wq = np.asarray(Wq, np.float32)
    Wk = np.asarray(Wk, np.float32)
    Wv = np.asarray(Wv, np.float32)
    Wo = np.asarray(Wo, np.float32)
    bq = np.asarray(bq, np.float32)
    bk = np.asarray(bk, np.float32)
    bv = np.asarray(bv, np.float32)
    bo = np.asarray(bo, np.float32)
    sinks = np.asarray(sinks, np.float32)

    in_maps = []
    for c in range(N_CORES):
        wq_c = Wq[c * HD_L : (c + 1) * HD_L] * np.float32(SCALING)  # [512, E]
        wqT = np.zeros((EP, HD_L), np.float32)
        wqT[:E] = wq_c.T
        wq_sw = np.ascontiguousarray(
            wqT.reshape(KT, 128, HD_L).transpose(1, 0, 2).reshape(128, KT * HD_L)
        ).astype(bf)

        wkv_c = np.concatenate(
            [Wk[c * D : (c + 1) * D], Wv[c * D : (c + 1) * D]], axis=0
        )  # [128, E]
        wkvT = np.zeros((EP, 128), np.float32)
        wkvT[:E] = wkv_c.T
        wkv_sw = np.ascontiguousarray(
            wkvT.reshape(KT, 128, 128).transpose(1, 0, 2).reshape(128, KT * 128)
        ).astype(bf)

        woT = np.ascontiguousarray(Wo[c * EC : (c + 1) * EC, :].T)  # [4096, 360]
        wo_sw = np.ascontiguousarray(
            woT.reshape(KO, 128, EC).transpose(1, 0, 2).reshape(128, KO * EC)
        ).astype(bf)

        bq_c = (bq[c * HD_L : (c + 1) * HD_L] * np.float32(SCALING)).reshape(4, 128)
        bq_dev = np.ascontiguousarray(bq_c.T)  # [128, 4]
        bkv_dev = np.ascontiguousarray(
            np.concatenate([bk[c * D : (c + 1) * D], bv[c * D : (c + 1) * D]]).reshape(
                128, 1
            )
        )
        bo_dev = np.ascontiguousarray(
            np.broadcast_to(bo[c * EC : (c + 1) * EC].reshape(1, EC), (128, EC))
        )
        esink8 = np.ascontiguousarray(
            np.exp(sinks[c * HL : (c + 1) * HL]).astype(np.float32).reshape(8, 1)
        )

        in_maps.append(
            {
                "x": x_sw,
                "wq": wq_sw,
                "wkv": wkv_sw,
                "wo": wo_sw,
                "bq": bq_dev,
                "bkv": bkv_dev,
                "bo": bo_dev,
                "cos2": cos2,
                "sin2s": sin2s,
                "maskAB": maskAB,
                "esink8": esink8,
                "sel": sel,
                "prot": np.ascontiguousarray(prot),
                "pkd": np.ascontiguousarray(pkd),
                "pkr": np.ascontiguousarray(pkr),
            }
        )
    return in_maps


# ------------------------------------------------------------- device build
def build_program():
    _install_tile_compat()
    _install_prof_shim()
    dbg_no_cc = os.environ.get("DBG_NO_CC") == "1"
    dbg_no_actdma = os.environ.get("DBG_NO_ACTDMA") == "1"
    dbg_no_gpdma = os.environ.get("DBG_NO_GPDMA") == "1"
    dbg_no_memset = os.environ.get("DBG_NO_MEMSET") == "1"
    dbg_phase = int(os.environ.get("DBG_PHASE", "4"))

    nc = bass.Bass("TRN2", target_bir_lowering=False, debug=False, num_devices=N_CORES)

    x_d = nc.declare_dram_parameter("x", [128, NQW * KT * 256], BF16, isOutput=False)
    wq_d = nc.declare_dram_parameter("wq", [128, KT * HD_L], BF16, isOutput=False)
    wkv_d = nc.declare_dram_parameter("wkv", [128, KT * 128], BF16, isOutput=False)
    wo_d = nc.declare_dram_parameter("wo", [128, KO * EC], BF16, isOutput=False)
    bq_d = nc.declare_dram_parameter("bq", [128, 4], FP32, isOutput=False)
    bkv_d = nc.declare_dram_parameter("bkv", [128, 1], FP32, isOutput=False)
    bo_d = nc.declare_dram_parameter("bo", [128, EC], FP32, isOutput=False)
    cos_d = nc.declare_dram_parameter("cos2", [128, S], FP32, isOutput=False)
    sin_d = nc.declare_dram_parameter("sin2s", [128, S], FP32, isOutput=False)
    mask_d = nc.declare_dram_parameter("maskAB", [128, 1536], BF16, isOutput=False)
    esink_d = nc.declare_dram_parameter("esink8", [8, 1], FP32, isOutput=False)
    sel_d = nc.declare_dram_parameter("sel", [8, NQT * 128], FP32R, isOutput=False)
    prot_d = nc.declare_dram_parameter("prot", [128, 128], FP32R, isOutput=False)
    pkd_d = nc.declare_dram_parameter("pkd", [64, 128], FP32R, isOutput=False)
    pkr_d = nc.declare_dram_parameter("pkr", [64, 128], FP32R, isOutput=False)
    y = nc.declare_dram_parameter("y", [S, EC], FP32, isOutput=True)

    Ident = mybir.ActivationFunctionType.Identity
    Exp = mybir.ActivationFunctionType.Exp
    Mult = mybir.AluOpType.mult
    Add = mybir.AluOpType.add

    with tile.TileContext(nc) as tc, nc.allow_low_precision(
        reason="bf16/fp32r operands for PE fast path; accumulation stays fp32"
    ):
        with (
            tc.tile_pool(name="persist", bufs=1) as per,
            tc.tile_pool(name="qb_pool", bufs=3) as qbp,
            tc.tile_pool(name="kvb_pool", bufs=2) as kvbp,
            tc.tile_pool(name="tmp_pool", bufs=3) as tmpp,
            tc.tile_pool(name="pp_pool", bufs=6) as ppp,
            tc.tile_pool(name="d8_pool", bufs=2) as d8p,
            tc.tile_pool(name="r8_pool", bufs=2) as r8p,
            tc.tile_pool(name="at_pool", bufs=8) as atp,
            tc.tile_pool(name="out_pool", bufs=2) as outp,
            tc.tile_pool(name="pj_ps", bufs=2, space="PSUM") as pj,
            tc.tile_pool(name="sc_ps", bufs=4, space="PSUM") as scp,
            tc.tile_pool(name="pv_ps", bufs=2, space="PSUM") as pvp,
            tc.tile_pool(name="dram", bufs=2, space="DRAM") as dram,
        ):
            # ------------------------------------------------ persistent SBUF
            x_all = per.tile([128, NQW * KT * 256], BF16)
            wq_sb = per.tile([128, KT * HD_L], BF16)
            wkv_sb = per.tile([128, KT * 128], BF16)
            wo_sb = per.tile([128, KO * EC], BF16)
            cos_sb = per.tile([128, S], FP32)
            sin_sb = per.tile([128, S], FP32)
            mask_sb = per.tile([128, 1536], BF16)
            qT = per.tile([128, NQT, S], BF16)
            k2T = per.tile([128, S], BF16)
            v_sb = per.tile([128, 8, 66], BF16)
            attnT = per.tile([128, NQT, S], BF16)
            den_seg = per.tile([65, 2048], FP32)
            bq_sb = per.tile([128, 4], FP32)
            bkv_sb = per.tile([128, 1], FP32)
            bo_sb = per.tile([128, EC], FP32)
            esink_sb = per.tile([8, 1], FP32)
            sel_sb = per.tile([8, NQT * 128], FP32R)
            prot_sb = per.tile([128, 128], FP32R)
            pkd_sb = per.tile([64, 128], FP32R)
            pkr_sb = per.tile([64, 128], FP32R)
            ident = per.tile([128, 128], FP32)

            ag_space = {} if dbg_no_cc else {"addr_space": "Shared"}
            ag_out = [
                dram.tile([H * D, 256], BF16, name=f"ag_out{w}", **ag_space)
                for w in range(NQW)
            ]

            # ------------------------------------------- initial loads
            # sync queue: consts then wq/x(w0) interleaved, wkv, x(w1..3)
            nc.sync.dma_start(bq_sb[:], bq_d[:])
            nc.sync.dma_start(bkv_sb[:], bkv_d[:])
            nc.sync.dma_start(esink_sb[:], esink_d[:])
            nc.sync.dma_start(sel_sb[:], sel_d[:])
            nc.sync.dma_start(prot_sb[:], prot_d[:])
            nc.sync.dma_start(pkd_sb[:], pkd_d[:])
            nc.sync.dma_start(pkr_sb[:], pkr_d[:])
            # sync queue: wkv + x (the critical path into kv_group/proj(w0))
            nc.sync.dma_start(wkv_sb[:], wkv_d[:])
            kch = [(0, 6), (6, 12), (12, 18), (18, 23)]
            for k0, k1 in kch:
                nc.sync.dma_start(
                    x_all[:, k0 * 256 : k1 * 256], x_d[:, k0 * 256 : k1 * 256]
                )
            for w in range(1, NQW):
                nc.sync.dma_start(
                    x_all[:, w * XW : (w + 1) * XW], x_d[:, w * XW : (w + 1) * XW]
                )
            # scalar(ACT) queue: wq (parallel with sync's wkv/x stream)
            _eng_t = nc.sync if dbg_no_actdma else nc.scalar
            for k0, k1 in kch:
                _eng_t.dma_start(
                    wq_sb[:, k0 * HD_L : k1 * HD_L], wq_d[:, k0 * HD_L : k1 * HD_L]
                )
            # gpsimd queue: dummy collective to absorb first-CC setup cost,
            # then tables and o_proj weights (needed only from ~mid-kernel)
            _eng_g = nc.sync if dbg_no_gpdma else nc.gpsimd
            if not dbg_no_cc:
                warm_in = dram.tile([128, 2], BF16, name="warm_in", tag="warm_in")
                warm_out = dram.tile(
                    [128 * N_CORES, 2], BF16, name="warm_out", addr_space="Shared"
                )
                nc.gpsimd.dma_start(warm_in[:], x_d[:, 0:2])
                nc.gpsimd.collective_compute(
                    "AllGather",
                    mybir.AluOpType.bypass,
                    ins=[warm_in[:].opt()],
                    outs=[warm_out[:].opt()],
                    replica_groups=[list(range(N_CORES))],
                )
            _eng_g.dma_start(cos_sb[:], cos_d[:])
            _eng_g.dma_start(sin_sb[:], sin_d[:])
            _eng_g.dma_start(mask_sb[:], mask_d[:])
            _eng_g.dma_start(bo_sb[:], bo_d[:])
            _eng_g.dma_start(wo_sb[:, : 16 * EC], wo_d[:, : 16 * EC])
            _eng_g.dma_start(wo_sb[:, 16 * EC :], wo_d[:, 16 * EC :])

            make_identity(nc, ident[:])
            if not dbg_no_memset:
                nc.vector.memset(v_sb[:, :, 64:65], 1.0)

            d8_h = {}

            # ------------------------------------------------ window helpers
            def kv_group(w):
                ps = pj.tile([128, 256], FP32, name="pjkv", tag="pj")
                for k in range(KT):
                    nc.tensor.matmul(
                        ps[:],
                        wkv_sb[:, k * 128 : (k + 1) * 128],
                        x_all[:, w * XW + k * 256 : w * XW + (k + 1) * 256],
                        start=k == 0,
                        stop=k == KT - 1,
                    )
                kvb = kvbp.tile([128, 256], FP32R, name="kvb", tag="kvb")
                nc.scalar.activation(kvb[:], ps[:], Ident, bias=bkv_sb[:, 0:1])
                qsl = slice(w * 256, (w + 1) * 256)
                kd = pj.tile([128, 256], FP32, name="kd", tag="pj")
                nc.tensor.matmul(
                    kd[:], pkd_sb[:], kvb[0:64, :], start=True, stop=True
                )
                kr = pj.tile([128, 256], FP32, name="kr", tag="pj")
                nc.tensor.matmul(
                    kr[:], pkr_sb[:], kvb[0:64, :], start=True, stop=True
                )
                ktmp = tmpp.tile([128, 256], BF16, name="ktmp", tag="tmp")
                nc.vector.tensor_tensor(ktmp[:], kr[:], sin_sb[:, qsl], Mult)
                nc.vector.tensor_tensor(k2T[:, qsl], kd[:], cos_sb[:, qsl], Mult)
                nc.vector.tensor_tensor(k2T[:, qsl], k2T[:, qsl], ktmp[:], Add)
                for b in range(2):
                    vt = pj.tile([128, 64], FP32, name="vt", tag="pj")
                    nc.tensor.transpose(
                        vt[:],
                        kvb[64:128, b * 128 : (b + 1) * 128].bitcast(FP32),
                        ident[64:128, 64:128],
                    )
                    nc.vector.tensor_copy(v_sb[:, 2 * w + b, 0:64], vt[:])

            def proj_rope_t(w, t):
                qsl = slice(w * 256, (w + 1) * 256)
                ps = pj.tile([128, 256], FP32, name="pjq", tag="pj")
                for k in range(KT):
                    nc.tensor.matmul(
                        ps[:],
                        wq_sb[:, k * HD_L + t * 128 : k * HD_L + (t + 1) * 128],
                        x_all[:, w * XW + k * 256 : w * XW + (k + 1) * 256],
                        start=k == 0,
                        stop=k == KT - 1,
                    )
                qb = qbp.tile([128, 256], FP32R, name="qb", tag="qb")
                nc.scalar.activation(qb[:], ps[:], Ident, bias=bq_sb[:, t : t + 1])
                rot = pj.tile([128, 256], FP32, name="rot", tag="pj")
                nc.tensor.matmul(
                    rot[:], prot_sb[:], qb[:], start=True, stop=True
                )
                qTs = qT[:, t, qsl]
                qtmp = tmpp.tile([128, 256], BF16, name="qtmp", tag="tmp")
                nc.vector.tensor_tensor(qtmp[:], rot[:], sin_sb[:, qsl], Mult)
                nc.vector.tensor_tensor(qTs, qb[:], cos_sb[:, qsl], Mult)
                nc.vector.tensor_tensor(qTs, qTs, qtmp[:], Add)

            def attn(w):
                s0 = 1 if w == 0 else 0
                for t in range(NQT):
                    pv = pvp.tile([65, 512], FP32, name="pv", tag="pv")
                    qsl = slice(w * 256, (w + 1) * 256)
                    for slot in range(s0, 3):
                        kb = 2 * w - 1 + slot
                        ksl = slice(kb * 128, (kb + 1) * 128)
                        scA = scp.tile([128, 256], FP32, name="scA", tag="sc")
                        nc.tensor.matmul(
                            scA[:], k2T[0:64, ksl], qT[0:64, t, qsl],
                            start=True, stop=True,
                        )
                        scB = scp.tile([128, 256], FP32, name="scB", tag="sc")
                        nc.tensor.matmul(
                            scB[:], k2T[64:128, ksl], qT[64:128, t, qsl],
                            start=True, stop=True,
                        )
                        p = ppp.tile([128, 512], BF16, name="p", tag="pp")
                        nc.scalar.activation(p[:, 0:256], scA[:], Exp)
                        nc.scalar.activation(p[:, 256:512], scB[:], Exp)
                        nc.vector.tensor_tensor(
                            p[:], p[:], mask_sb[:, slot * 512 : (slot + 1) * 512], Mult
                        )
                        nc.tensor.matmul(
                            pv[:], v_sb[:, kb, 0:65], p[:],
                            start=slot == s0, stop=slot == 2,
                        )
                    nc.vector.tensor_copy(attnT[0:64, t, qsl], pv[0:64, 0:256])
                    nc.vector.tensor_copy(attnT[64:128, t, qsl], pv[0:64, 256:512])
                    nc.scalar.activation(
                        den_seg[64:65, t * 512 : (t + 1) * 512],
                        pv[64:65, 0:512], Ident,
                    )
                d8 = d8p.tile([8, 256], FP32, name="d8", tag="d8")
                (nc.sync if dbg_no_actdma else nc.scalar).dma_start(
                    d8[:], den_seg[64:65, :]
                )
                d8_h[w] = d8

            def renorm_ship(w):
                qsl = slice(w * 256, (w + 1) * 256)
                d8 = d8_h.pop(w)
                nc.vector.tensor_scalar(d8[:], d8[:], esink_sb[:, 0:1], None, Add)
                r8 = r8p.tile([8, 256], FP32R, name="r8", tag="r8")
                nc.vector.reciprocal(r8[:], d8[:])
                for t in range(NQT):
                    bc = scp.tile([128, 256], FP32, name="bc", tag="sc")
                    nc.tensor.matmul(
                        bc[:], sel_sb[:, t * 128 : (t + 1) * 128], r8[:],
                        start=True, stop=True,
                    )
                    nc.vector.tensor_tensor(
                        attnT[:, t, qsl], attnT[:, t, qsl], bc[:], Mult
                    )
                ag_in = dram.tile([HD_L, 256], BF16, name="ag_in", tag="ag_in")
                (nc.sync if dbg_no_gpdma else nc.gpsimd).dma_start(
                    ag_in[:].rearrange("(t p) s -> p t s", p=128),
                    attnT[:, :, qsl],
                )
                if dbg_no_cc:
                    for cc in range(N_CORES):
                        nc.sync.dma_start(
                            ag_out[w][cc * HD_L : (cc + 1) * HD_L, :], ag_in[:]
                        )
                else:
                    nc.gpsimd.collective_compute(
                        "AllGather",
                        mybir.AluOpType.bypass,
                        ins=[ag_in[:].opt()],
                        outs=[ag_out[w][:].opt()],
                        replica_groups=[list(range(N_CORES))],
                    )

            def oproj(w):
                ats = []
                for cch in range(4):
                    at = atp.tile([128, 2048], BF16, name=f"at{cch}", tag="at")
                    eng = nc.sync if (cch % 2 == 0 or dbg_no_actdma) else nc.scalar
                    eng.dma_start(
                        at[:],
                        ag_out[w][cch * 1024 : (cch + 1) * 1024, :].rearrange(
                            "(k p) s -> p k s", p=128
                        ),
                    )
                    ats.append(at)
                for i in range(2):
                    po = pvp.tile([128, EC], FP32, name="po", tag="pv")
                    for k in range(KO):
                        at = ats[k // 8]
                        kk = k % 8
                        nc.tensor.matmul(
                            po[:],
                            at[:, kk * 256 + i * 128 : kk * 256 + i * 128 + 128],
                            wo_sb[:, k * EC : (k + 1) * EC],
                            start=k == 0,
                            stop=k == KO - 1,
                        )
                    os_ = outp.tile([128, EC], FP32, name="os", tag="out")
                    nc.vector.tensor_tensor(os_[:], po[:], bo_sb[:], Add)
                    (nc.sync if dbg_no_gpdma else nc.gpsimd).dma_start(
                        y[(2 * w + i) * 128 : (2 * w + i + 1) * 128, :], os_[:]
                    )

            # ------------------------------------------------ the pipeline
            for w in range(NQW):
                kv_group(w)
                for t in range(NQT):
                    proj_rope_t(w, t)
                if dbg_phase >= 2:
                    attn(w)
                if w >= 1 and dbg_phase >= 4:
                    oproj(w - 1)
                if dbg_phase >= 3:
                    renorm_ship(w)
            if dbg_phase >= 4:
                oproj(NQW - 1)
            else:
                src_t = attnT if dbg_phase >= 2 else qT
                for sb in range(8):
                    os_ = outp.tile([128, EC], FP32, name="osd", tag="out")
                    nc.vector.tensor_copy(os_[:], src_t[:, 0, sb : sb + EC])
                    nc.sync.dma_start(y[sb * 128 : (sb + 1) * 128, :], os_[:])

    return nc


_PROGRAM = [None]


def _get_program():
    if _PROGRAM[0] is None:
        _PROGRAM[0] = build_program()
    return _PROGRAM[0]


def kernel(**inputs) -> np.ndarray:
    nc = _get_program()
    in_maps = host_prepare(**inputs)
    res = run_bass_kernel_spmd(nc, in_maps, list(range(N_CORES)))
    out = np.concatenate([res.results[c]["y"] for c in range(N_CORES)], axis=1)
    return out.reshape(B, S, E)


def kernel_traced(tmpdir=None, **inputs):
    """Like kernel() but with NTFF profiling; returns (out, BassKernelResults)."""
    _install_prof_shim()
    from concourse import bass_utils

    bass_utils.upload_artifacts = lambda d: str(d)
    nc = _get_program()
    in_maps = host_prepare(**inputs)
    res = run_bass_kernel_spmd(
        nc, in_maps, list(range(N_CORES)), trace=True, tmpdir=tmpdir
    )
    out = np.concatenate([res.results[c]["y"] for c in range(N_CORES)], axis=1)
    return out.reshape(B, S, E), res


# revision 23
# speedup vs baseline: 1.0297x; 1.0297x over previous
"""Trainium2 Bass kernel for nn_Attention_38697655337033 (sparse_attention).

GPT-OSS-style sliding-window attention block: QKV proj + YaRN RoPE + GQA
(64 Q heads / 8 KV heads, D=64, window 128, causal) + attention sinks +
o_proj.  Sharded over 8 NeuronCores tensor-parallel by head: core c owns
query heads 8c..8c+7 and KV head c.  o_proj is column-parallel over the
2880 output features (360 per core) after an AllGather of the per-core
attention outputs, chunked by 256-query window so collectives overlap
the next window's compute.

This version is a fully pipelined single-scope schedule:
  - all DRAM tensors are host-swizzled so every DMA is contiguous per
    partition (few, large DMAs; ~350 GB/s per transfer),
  - per-window pipeline: proj(w) -> rope(w) -> attn(w) -> renorm/AG(w),
    with renorm(w-1) broadcast matmuls and oproj(w-1) interleaved so the
    PE never waits on collectives until the tail,
  - rope rotate-half runs on the PE via permutation matmuls (fp32r),
  - scores A/B head-halves run concurrently in disjoint PE row-quadrants,
  - DMA issue is spread across the Sync, Activation and GpSimd queues.

Numerics: projections, scores/PV and o_proj run bf16 operands with fp32
PSUM accumulation; rope runs fp32(r); softmax has no max-subtraction and
folds the sink into the denominator (fp32).

Self-contained: hardcodes all shapes; builds and caches the Bass program
on first call.
"""

import math
import os
import sys
import types

import numpy as np

try:
    import concourse.bass as bass  # noqa: F401
except ImportError:  # pragma: no cover
    sys.path.insert(0, "/opt/trn_rl_repo")

import ml_dtypes

import concourse.bass as bass
import concourse.mybir as mybir
import concourse.tile as tile
from concourse.bass_utils import run_bass_kernel_spmd
from concourse.masks import make_identity
from concourse.tile import ScopedClock

# ---------------------------------------------------------------- constants
B, S, E = 1, 1024, 2880
H, KV, D = 64, 8, 64
WIN = 128
BASE, SCALE, ORIG = 150000.0, 32.0, 4096
BFAST, BSLOW = 32.0, 1.0
SCALING = D ** -0.5  # 0.125, exact power of two -> folded into Wq on host

N_CORES = 8
HL = H // N_CORES          # 8 local query heads
HD_L = HL * D              # 512 local q dims
EC = E // N_CORES          # 360 output columns per core
EP = 2944                  # E padded to 23*128
KT = EP // 128             # 23 contraction tiles for projections
NQT = HL // 2              # 4 head-pair tiles
NQW = S // 256             # 4 query windows of 256
KO = H * D // 128          # 32 o_proj contraction tiles
XW = KT * 256              # 5888 x columns per window

FP32 = mybir.dt.float32
FP32R = mybir.dt.float32r
BF16 = mybir.dt.bfloat16

# ------------------------------------------------------- walrus compat patch
# This container's walrus build rejects instructions with >1 sync-wait
# ("Too many sync wait commands").  Split extra waits onto same-engine NoOp
# carriers, and split the final Tile drain into one drain per wait.
_compat_done = [False]
_carrier_n = [0]


def _install_tile_compat():
    if _compat_done[0]:
        return
    _compat_done[0] = True

    orig_cal = tile.TileContext._commit_and_lower

    def patched_cal(self, inst, original_block, old_bb_map, bb_to_exit_bb):
        if isinstance(inst, mybir.Instruction):
            si = getattr(inst, "sync_info", None)
            if si is not None and len(si.on_wait) > 1:
                waits = list(si.on_wait)
                for w in waits[:-1]:
                    _carrier_n[0] += 1
                    nop = mybir.InstNoOp(
                        name=f"swsplit-{_carrier_n[0]}",
                        engine=inst.engine,
                        sync_info=mybir.SyncInfo(on_wait=[w], on_update=[]),
                        bass_nofuse=True,
                    )
                    self._commit_instruction(nop)
                inst.sync_info = mybir.SyncInfo(
                    on_wait=[waits[-1]], on_update=list(si.on_update)
                )
        return orig_cal(self, inst, original_block, old_bb_map, bb_to_exit_bb)

    tile.TileContext._commit_and_lower = patched_cal

    def patched_dab(self, tick_clock, wait_clock):
        drain_inst = self.nc.sync.drain()
        wait_clock.add_sem_waits(
            drain_inst.ins, ScopedClock({None: tick_clock.global_clock})
        )
        si = drain_inst.ins.sync_info
        if si is not None and len(si.on_wait) > 1:
            waits = list(si.on_wait)
            drain_inst.ins.sync_info = mybir.SyncInfo(on_wait=waits[:1], on_update=[])
            for i in range(1, len(waits)):
                extra = self.nc.sync.drain()
                extra.ins.sync_info = mybir.SyncInfo(
                    on_wait=waits[i : i + 1], on_update=[]
                )
        self.nc.all_engine_barrier()
        assert self.sems is not None
        popped = self.nc._tile_sem_poison_stack.pop()
        assert popped is self._sem_poison
        self.nc.clear_and_free_semaphores(list(self.sems.allocated().values()))
        self.nc.all_engine_barrier()

    tile.TileContext._drain_and_barrier = patched_dab


def _install_ldw_opt():
    """Enable walrus LDWEIGHTS optimization (pull-ahead/FWL) — the stock
    compile command pins it off, which serializes a ~107ns weight load in
    front of every matmul."""
    from concourse import bass_utils

    if getattr(bass_utils, "_ldw_patched", False):
        return
    orig = bass_utils.run_command

    def patched(cmd, *a, **k):
        if isinstance(cmd, list):
            cmd = [
                "--enable-ldw-opt=true" if c == "--enable-ldw-opt=false" else c
                for c in cmd
            ]
        return orig(cmd, *a, **k)

    bass_utils.run_command = patched
    bass_utils._ldw_patched = True


def _install_prof_shim():
    """antenv.axon_hooks is missing in this container; provide it so
    BASS_TRACE-style profiling paths don't crash."""
    try:
        import antenv.axon_hooks  # noqa: F401
        return
    except ImportError:
        pass
    try:
        import antenv
        from trn_agent_boot.trn_boot import _ntff_profile_via_ctypes

        hook = _ntff_profile_via_ctypes("/opt/axon/libaxon_pjrt.so")
    except Exception:
        hook = None
        try:
            import antenv
        except ImportError:
            return
    mod = types.ModuleType("antenv.axon_hooks")
    mod._hook = hook
    mod.get_axon_ntff_profile_hook = lambda: mod._hook

    def _set(h):
        mod._hook = h

    mod.set_axon_ntff_profile_hook = _set
    sys.modules["antenv.axon_hooks"] = mod
    antenv.axon_hooks = mod


# ---------------------------------------------------------------- host prep
def _rope_tables_np(positions):
    """cos/sin YaRN tables, mirroring the reference, in float32."""
    def find_dim(rot):
        return D * math.log(ORIG / (rot * 2 * math.pi)) / (2 * math.log(BASE))

    low = max(find_dim(BFAST), 0.0)
    high = min(find_dim(BSLOW), D // 2 - 1)
    if low == high:
        high += 0.001
    pos_freqs = (BASE ** (np.arange(0, D, 2, dtype=np.float32) / np.float32(D))).astype(
        np.float32
    )
    ramp = np.clip(
        (np.arange(D // 2, dtype=np.float32) - np.float32(low))
        / np.float32(high - low),
        0.0,
        1.0,
    ).astype(np.float32)
    inv_freq = (
        (np.float32(1.0) / (np.float32(SCALE) * pos_freqs)) * ramp
        + (np.float32(1.0) / pos_freqs) * (np.float32(1.0) - ramp)
    ).astype(np.float32)
    mscale = np.float32(0.1 * math.log(SCALE) + 1.0)
    ang = positions.astype(np.float32)[:, None] * inv_freq[None, :]  # [S, 32]
    emb = np.concatenate([ang, ang], axis=-1)  # [S, D]
    return (np.cos(emb) * mscale).astype(np.float32), (np.sin(emb) * mscale).astype(
        np.float32
    )


def _make_masks():
    """Multiplicative [128, 256] masks per slot in the transposed-score
    layout.  Slot s (of 3) covers key block kb = 2Q-1+s for query window Q
    (256 wide).  Entry [j, i2] is 1 when query i2 may attend key j of that
    block:
      slot0: i2 <  j           (keys one block behind the window)
      slot1: j <= i2 <= j+127  (keys in the window's first block)
      slot2: i2 >= j+128       (keys in the window's second block)
    Window 0 simply skips slot 0 (its key block doesn't exist)."""
    j = np.arange(128)[:, None]
    i2 = np.arange(256)[None, :]
    m0 = (i2 < j).astype(np.float32)
    m1 = ((i2 >= j) & (i2 <= j + 127)).astype(np.float32)
    m2 = (i2 >= j + 128).astype(np.float32)
    return m0, m1, m2


def _swap64(m):
    return (m // 64) * 64 + (m % 64 + 32) % 64


def host_prepare(hidden_states, positions, Wq, bq, Wk, bk, Wv, bv, Wo, bo, sinks):
    """Build the 8 per-core input maps (all partition-contiguous layouts)."""
    bf = ml_dtypes.bfloat16
    x = np.asarray(hidden_states, np.float32).reshape(S, E)
    xT = np.zeros((EP, S), np.float32)
    xT[:E] = x.T
    # [p, w, k, s2] flattened to [128, NQW*KT*256]
    x_sw = np.ascontiguousarray(
        xT.reshape(KT, 128, NQW, 256).transpose(1, 2, 0, 3).reshape(128, NQW * XW)
    ).astype(bf)

    cos, sin = _rope_tables_np(np.asarray(positions))
    cosT = np.ascontiguousarray(cos.T)  # [64, S]
    sinT = np.ascontiguousarray(sin.T)
    sgn = np.where(np.arange(D) < D // 2, np.float32(-1.0), np.float32(1.0))
    sinTs = sinT * sgn[:, None]
    cos2 = np.ascontiguousarray(np.concatenate([cosT, cosT], axis=0))  # [128, S]
    sin2s = np.ascontiguousarray(np.concatenate([sinTs, sinTs], axis=0))

    m0, m1, m2 = _make_masks()
    maskAB = np.ascontiguousarray(
        np.concatenate([np.concatenate([m, m], axis=1) for m in (m0, m1, m2)], axis=1)
    ).astype(bf)  # [128, 1536]

    # permutation matrices for rope rotate-half / k duplication (fp32r)
    mm_ = np.arange(128)
    prot = np.zeros((128, 128), np.float32)
    prot[_swap64(mm_), mm_] = 1.0
    pkd = np.zeros((64, 128), np.float32)
    pkd[mm_ % 64, mm_] = 1.0
    pkr = np.zeros((64, 128), np.float32)
    pkr[(mm_ % 64 + 32) % 64, mm_] = 1.0

    # renorm broadcast row-selectors: sel[r, t*128+p] = 1 iff r == 2t + p//64
    sel = np.zeros((8, NQT, 128), np.float32)
    for t in range(NQT):
        sel[2 * t, t, 0:64] = 1.0
        sel[2 * t + 1, t, 64:128] = 1.0
    sel = np.ascontiguousarray(sel.reshape(8, NQT * 128))

    Wq = np.asarray(Wq, np.float32)
    Wk = np.asarray(Wk, np.float32)
    Wv = np.asarray(Wv, np.float32)
    Wo = np.asarray(Wo, np.float32)
    bq = np.asarray(bq, np.float32)
    bk = np.asarray(bk, np.float32)
    bv = np.asarray(bv, np.float32)
    bo = np.asarray(bo, np.float32)
    sinks = np.asarray(sinks, np.float32)

    in_maps = []
    for c in range(N_CORES):
        wq_c = Wq[c * HD_L : (c + 1) * HD_L] * np.float32(SCALING)  # [512, E]
        wqT = np.zeros((EP, HD_L), np.float32)
        wqT[:E] = wq_c.T
        wq_sw = np.ascontiguousarray(
            wqT.reshape(KT, 128, HD_L).transpose(1, 0, 2).reshape(128, KT * HD_L)
        ).astype(bf)

        wkv_c = np.concatenate(
            [Wk[c * D : (c + 1) * D], Wv[c * D : (c + 1) * D]], axis=0
        )  # [128, E]
        wkvT = np.zeros((EP, 128), np.float32)
        wkvT[:E] = wkv_c.T
        wkv_sw = np.ascontiguousarray(
            wkvT.reshape(KT, 128, 128).transpose(1, 0, 2).reshape(128, KT * 128)
        ).astype(bf)

        woT = np.ascontiguousarray(Wo[c * EC : (c + 1) * EC, :].T)  # [4096, 360]
        wo_sw = np.ascontiguousarray(
            woT.reshape(KO, 128, EC).transpose(1, 0, 2).reshape(128, KO * EC)
        ).astype(bf)

        bq_c = (bq[c * HD_L : (c + 1) * HD_L] * np.float32(SCALING)).reshape(4, 128)
        bq_dev = np.ascontiguousarray(bq_c.T)  # [128, 4]
        bkv_dev = np.ascontiguousarray(
            np.concatenate([bk[c * D : (c + 1) * D], bv[c * D : (c + 1) * D]]).reshape(
                128, 1
            )
        )
        bo_dev = np.ascontiguousarray(
            np.broadcast_to(bo[c * EC : (c + 1) * EC].reshape(1, EC), (128, EC))
        )
        esink8 = np.ascontiguousarray(
            np.exp(sinks[c * HL : (c + 1) * HL]).astype(np.float32).reshape(8, 1)
        )

        in_maps.append(
            {
                "x": x_sw,
                "wq": wq_sw,
                "wkv": wkv_sw,
                "wo": wo_sw,
                "bq": bq_dev,
                "bkv": bkv_dev,
                "bo": bo_dev,
                "cos2": cos2,
                "sin2s": sin2s,
                "maskAB": maskAB,
                "esink8": esink8,
                "sel": sel,
                "prot": np.ascontiguousarray(prot),
                "pkd": np.ascontiguousarray(pkd),
                "pkr": np.ascontiguousarray(pkr),
            }
        )
    return in_maps


# ------------------------------------------------------------- device build
def build_program():
    _install_tile_compat()
    _install_prof_shim()
    dbg_no_cc = os.environ.get("DBG_NO_CC") == "1"
    dbg_no_actdma = os.environ.get("DBG_NO_ACTDMA") == "1"
    dbg_no_gpdma = os.environ.get("DBG_NO_GPDMA") == "1"
    dbg_no_memset = os.environ.get("DBG_NO_MEMSET") == "1"
    dbg_phase = int(os.environ.get("DBG_PHASE", "4"))

    nc = bass.Bass("TRN2", target_bir_lowering=False, debug=False, num_devices=N_CORES)

    x_d = nc.declare_dram_parameter("x", [128, NQW * KT * 256], BF16, isOutput=False)
    wq_d = nc.declare_dram_parameter("wq", [128, KT * HD_L], BF16, isOutput=False)
    wkv_d = nc.declare_dram_parameter("wkv", [128, KT * 128], BF16, isOutput=False)
    wo_d = nc.declare_dram_parameter("wo", [128, KO * EC], BF16, isOutput=False)
    bq_d = nc.declare_dram_parameter("bq", [128, 4], FP32, isOutput=False)
    bkv_d = nc.declare_dram_parameter("bkv", [128, 1], FP32, isOutput=False)
    bo_d = nc.declare_dram_parameter("bo", [128, EC], FP32, isOutput=False)
    cos_d = nc.declare_dram_parameter("cos2", [128, S], FP32, isOutput=False)
    sin_d = nc.declare_dram_parameter("sin2s", [128, S], FP32, isOutput=False)
    mask_d = nc.declare_dram_parameter("maskAB", [128, 1536], BF16, isOutput=False)
    esink_d = nc.declare_dram_parameter("esink8", [8, 1], FP32, isOutput=False)
    sel_d = nc.declare_dram_parameter("sel", [8, NQT * 128], FP32R, isOutput=False)
    prot_d = nc.declare_dram_parameter("prot", [128, 128], FP32R, isOutput=False)
    pkd_d = nc.declare_dram_parameter("pkd", [64, 128], FP32R, isOutput=False)
    pkr_d = nc.declare_dram_parameter("pkr", [64, 128], FP32R, isOutput=False)
    y = nc.declare_dram_parameter("y", [S, EC], FP32, isOutput=True)

    Ident = mybir.ActivationFunctionType.Identity
    Exp = mybir.ActivationFunctionType.Exp
    Mult = mybir.AluOpType.mult
    Add = mybir.AluOpType.add

    with tile.TileContext(nc) as tc, nc.allow_low_precision(
        reason="bf16/fp32r operands for PE fast path; accumulation stays fp32"
    ):
        with (
            tc.tile_pool(name="persist", bufs=1) as per,
            tc.tile_pool(name="qb_pool", bufs=3) as qbp,
            tc.tile_pool(name="kvb_pool", bufs=2) as kvbp,
            tc.tile_pool(name="tmp_pool", bufs=3) as tmpp,
            tc.tile_pool(name="pp_pool", bufs=6) as ppp,
            tc.tile_pool(name="d8_pool", bufs=2) as d8p,
            tc.tile_pool(name="r8_pool", bufs=2) as r8p,
            tc.tile_pool(name="at_pool", bufs=8) as atp,
            tc.tile_pool(name="out_pool", bufs=2) as outp,
            tc.tile_pool(name="pj_ps", bufs=2, space="PSUM") as pj,
            tc.tile_pool(name="sc_ps", bufs=4, space="PSUM") as scp,
            tc.tile_pool(name="pv_ps", bufs=2, space="PSUM") as pvp,
            tc.tile_pool(name="dram", bufs=2, space="DRAM") as dram,
        ):
            # ------------------------------------------------ persistent SBUF
            x_all = per.tile([128, NQW, KT, 256], BF16)
            wq_sb = per.tile([128, KT * HD_L], BF16)
            wkv_sb = per.tile([128, KT * 128], BF16)
            wo_sb = per.tile([128, KO * EC], BF16)
            cos_sb = per.tile([128, S], FP32)
            sin_sb = per.tile([128, S], FP32)
            mask_sb = per.tile([128, 1536], BF16)
            qT = per.tile([128, NQT, S], BF16)
            k2T = per.tile([128, S], BF16)
            v_sb = per.tile([128, 8, 66], BF16)
            attnT = per.tile([128, NQT, S], BF16)
            den_seg = per.tile([65, 2048], FP32)
            bq_sb = per.tile([128, 4], FP32)
            bkv_sb = per.tile([128, 1], FP32)
            bo_sb = per.tile([128, EC], FP32)
            esink_sb = per.tile([8, 1], FP32)
            sel_sb = per.tile([8, NQT * 128], FP32R)
            prot_sb = per.tile([128, 128], FP32R)
            pkd_sb = per.tile([64, 128], FP32R)
            pkr_sb = per.tile([64, 128], FP32R)
            ident = per.tile([128, 128], FP32)

            ag_space = {} if dbg_no_cc else {"addr_space": "Shared"}
            ag_out = [
                dram.tile([H * D, 256], BF16, name=f"ag_out{w}", **ag_space)
                for w in range(NQW)
            ]

            # ------------------------------------------- initial loads
            # sync queue: consts then wq/x(w0) interleaved, wkv, x(w1..3)
            nc.sync.dma_start(bq_sb[:], bq_d[:])
            nc.sync.dma_start(bkv_sb[:], bkv_d[:])
            nc.sync.dma_start(esink_sb[:], esink_d[:])
            nc.sync.dma_start(sel_sb[:], sel_d[:])
            nc.sync.dma_start(prot_sb[:], prot_d[:])
            nc.sync.dma_start(pkd_sb[:], pkd_d[:])
            nc.sync.dma_start(pkr_sb[:], pkr_d[:])
            # sync queue: wkv + x (the critical path into kv_group/proj(w0))
            nc.sync.dma_start(wkv_sb[:], wkv_d[:])
            kch = [(0, 6), (6, 12), (12, 18), (18, 23)]
            for k0, k1 in kch:
                nc.sync.dma_start(
                    x_all[:, 0, k0:k1, :], x_d[:, k0 * 256 : k1 * 256]
                )
            for w in range(1, NQW):
                nc.sync.dma_start(
                    x_all[:, w, :, :], x_d[:, w * XW : (w + 1) * XW]
                )
            # scalar(ACT) queue: wq (parallel with sync's wkv/x stream)
            _eng_t = nc.sync if dbg_no_actdma else nc.scalar
            for k0, k1 in kch:
                _eng_t.dma_start(
                    wq_sb[:, k0 * HD_L : k1 * HD_L], wq_d[:, k0 * HD_L : k1 * HD_L]
                )
            # gpsimd queue: dummy collective to absorb first-CC setup cost,
            # then tables and o_proj weights (needed only from ~mid-kernel)
            _eng_g = nc.sync if dbg_no_gpdma else nc.gpsimd
            if not dbg_no_cc:
                warm_in = dram.tile([128, 2], BF16, name="warm_in", tag="warm_in")
                warm_out = dram.tile(
                    [128 * N_CORES, 2], BF16, name="warm_out", addr_space="Shared"
                )
                nc.gpsimd.dma_start(warm_in[:], x_d[:, 0:2])
                nc.gpsimd.collective_compute(
                    "AllGather",
                    mybir.AluOpType.bypass,
                    ins=[warm_in[:].opt()],
                    outs=[warm_out[:].opt()],
                    replica_groups=[list(range(N_CORES))],
                )
            _eng_g.dma_start(cos_sb[:], cos_d[:])
            _eng_g.dma_start(sin_sb[:], sin_d[:])
            _eng_g.dma_start(mask_sb[:], mask_d[:])
            _eng_g.dma_start(bo_sb[:], bo_d[:])
            _eng_g.dma_start(wo_sb[:, : 16 * EC], wo_d[:, : 16 * EC])
            _eng_g.dma_start(wo_sb[:, 16 * EC :], wo_d[:, 16 * EC :])

            make_identity(nc, ident[:])
            if not dbg_no_memset:
                nc.vector.memset(v_sb[:, :, 64:65], 1.0)

            d8_h = {}

            # ------------------------------------------------ window helpers
            def kv_pair(p):
                ps = pj.tile([128, 512], FP32, name="pjkv", tag="pj")
                for k in range(KT):
                    nc.tensor.matmul(
                        ps[:],
                        wkv_sb[:, k * 128 : (k + 1) * 128],
                        x_all[:, 2 * p : 2 * p + 2, k, :],
                        start=k == 0,
                        stop=k == KT - 1,
                    )
                kvb = kvbp.tile([128, 512], FP32R, name="kvb", tag="kvb")
                nc.scalar.activation(kvb[:], ps[:], Ident, bias=bkv_sb[:, 0:1])
                psl = slice(p * 512, (p + 1) * 512)
                kd = pj.tile([128, 512], FP32, name="kd", tag="pj")
                nc.tensor.matmul(
                    kd[:], pkd_sb[:], kvb[0:64, :], start=True, stop=True
                )
                kr = pj.tile([128, 512], FP32, name="kr", tag="pj")
                nc.tensor.matmul(
                    kr[:], pkr_sb[:], kvb[0:64, :], start=True, stop=True
                )
                ktmp = tmpp.tile([128, 512], BF16, name="ktmp", tag="tmp")
                nc.vector.tensor_tensor(ktmp[:], kr[:], sin_sb[:, psl], Mult)
                nc.vector.tensor_tensor(k2T[:, psl], kd[:], cos_sb[:, psl], Mult)
                nc.vector.tensor_tensor(k2T[:, psl], k2T[:, psl], ktmp[:], Add)
                for b in range(4):
                    vt = pj.tile([128, 64], FP32, name="vt", tag="pj")
                    nc.tensor.transpose(
                        vt[:],
                        kvb[64:128, b * 128 : (b + 1) * 128].bitcast(FP32),
                        ident[64:128, 64:128],
                    )
                    nc.vector.tensor_copy(v_sb[:, 4 * p + b, 0:64], vt[:])

            def proj_rope_t(p, t):
                psl = slice(p * 512, (p + 1) * 512)
                ps = pj.tile([128, 512], FP32, name="pjq", tag="pj")
                for k in range(KT):
                    nc.tensor.matmul(
                        ps[:],
                        wq_sb[:, k * HD_L + t * 128 : k * HD_L + (t + 1) * 128],
                        x_all[:, 2 * p : 2 * p + 2, k, :],
                        start=k == 0,
                        stop=k == KT - 1,
                    )
                qb = qbp.tile([128, 512], FP32R, name="qb", tag="qb")
                nc.scalar.activation(qb[:], ps[:], Ident, bias=bq_sb[:, t : t + 1])
                rot = pj.tile([128, 512], FP32, name="rot", tag="pj")
                nc.tensor.matmul(
                    rot[:], prot_sb[:], qb[:], start=True, stop=True
                )
                qTs = qT[:, t, psl]
                qtmp = tmpp.tile([128, 512], BF16, name="qtmp", tag="tmp")
                nc.vector.tensor_tensor(qtmp[:], rot[:], sin_sb[:, psl], Mult)
                nc.vector.tensor_tensor(qTs, qb[:], cos_sb[:, psl], Mult)
                nc.vector.tensor_tensor(qTs, qTs, qtmp[:], Add)

            def attn(w):
                s0 = 1 if w == 0 else 0
                for t in range(NQT):
                    pv = pvp.tile([65, 512], FP32, name="pv", tag="pv")
                    qsl = slice(w * 256, (w + 1) * 256)
                    for slot in range(s0, 3):
                        kb = 2 * w - 1 + slot
                        ksl = slice(kb * 128, (kb + 1) * 128)
                        scA = scp.tile([128, 256], FP32, name="scA", tag="sc")
                        nc.tensor.matmul(
                            scA[:], k2T[0:64, ksl], qT[0:64, t, qsl],
                            start=True, stop=True,
                        )
                        scB = scp.tile([128, 256], FP32, name="scB", tag="sc")
                        nc.tensor.matmul(
                            scB[:], k2T[64:128, ksl], qT[64:128, t, qsl],
                            start=True, stop=True,
                        )
                        p = ppp.tile([128, 512], BF16, name="p", tag="pp")
                        nc.scalar.activation(p[:, 0:256], scA[:], Exp)
                        nc.scalar.activation(p[:, 256:512], scB[:], Exp)
                        nc.vector.tensor_tensor(
                            p[:], p[:], mask_sb[:, slot * 512 : (slot + 1) * 512], Mult
                        )
                        nc.tensor.matmul(
                            pv[:], v_sb[:, kb, 0:65], p[:],
                            start=slot == s0, stop=slot == 2,
                        )
                    nc.vector.tensor_copy(attnT[0:64, t, qsl], pv[0:64, 0:256])
                    nc.vector.tensor_copy(attnT[64:128, t, qsl], pv[0:64, 256:512])
                    nc.scalar.activation(
                        den_seg[64:65, t * 512 : (t + 1) * 512],
                        pv[64:65, 0:512], Ident,
                    )
                d8 = d8p.tile([8, 256], FP32, name="d8", tag="d8")
                (nc.sync if dbg_no_actdma else nc.scalar).dma_start(
                    d8[:], den_seg[64:65, :]
                )
                d8_h[w] = d8

            def renorm_ship(w):
                qsl = slice(w * 256, (w + 1) * 256)
                d8 = d8_h.pop(w)
                nc.vector.tensor_scalar(d8[:], d8[:], esink_sb[:, 0:1], None, Add)
                r8 = r8p.tile([8, 256], FP32R, name="r8", tag="r8")
                nc.vector.reciprocal(r8[:], d8[:])
                for t in range(NQT):
                    bc = scp.tile([128, 256], FP32, name="bc", tag="sc")
                    nc.tensor.matmul(
                        bc[:], sel_sb[:, t * 128 : (t + 1) * 128], r8[:],
                        start=True, stop=True,
                    )
                    nc.vector.tensor_tensor(
                        attnT[:, t, qsl], attnT[:, t, qsl], bc[:], Mult
                    )
                ag_in = dram.tile([HD_L, 256], BF16, name="ag_in", tag="ag_in")
                (nc.sync if dbg_no_gpdma else nc.gpsimd).dma_start(
                    ag_in[:].rearrange("(t p) s -> p t s", p=128),
                    attnT[:, :, qsl],
                )
                if dbg_no_cc:
                    for cc in range(N_CORES):
                        nc.sync.dma_start(
                            ag_out[w][cc * HD_L : (cc + 1) * HD_L, :], ag_in[:]
                        )
                else:
                    nc.gpsimd.collective_compute(
                        "AllGather",
                        mybir.AluOpType.bypass,
                        ins=[ag_in[:].opt()],
                        outs=[ag_out[w][:].opt()],
                        replica_groups=[list(range(N_CORES))],
                    )

            def oproj(w):
                ats = []
                for cch in range(4):
                    at = atp.tile([128, 2048], BF16, name=f"at{cch}", tag="at")
                    eng = nc.sync if (cch % 2 == 0 or dbg_no_actdma) else nc.scalar
                    eng.dma_start(
                        at[:],
                        ag_out[w][cch * 1024 : (cch + 1) * 1024, :].rearrange(
                            "(k p) s -> p k s", p=128
                        ),
                    )
                    ats.append(at)
                for i in range(2):
                    po = pvp.tile([128, EC], FP32, name="po", tag="pv")
                    for k in range(KO):
                        at = ats[k // 8]
                        kk = k % 8
                        nc.tensor.matmul(
                            po[:],
                            at[:, kk * 256 + i * 128 : kk * 256 + i * 128 + 128],
                            wo_sb[:, k * EC : (k + 1) * EC],
                            start=k == 0,
                            stop=k == KO - 1,
                        )
                    os_ = outp.tile([128, EC], FP32, name="os", tag="out")
                    nc.vector.tensor_tensor(os_[:], po[:], bo_sb[:], Add)
                    (nc.sync if dbg_no_gpdma else nc.gpsimd).dma_start(
                        y[(2 * w + i) * 128 : (2 * w + i + 1) * 128, :], os_[:]
                    )

            # ------------------------------------------------ the pipeline
            # pair 0: proj windows 0-1, their attention + shipping
            kv_pair(0)
            for t in range(NQT):
                proj_rope_t(0, t)
            if dbg_phase >= 2:
                attn(0)
                attn(1)
            if dbg_phase >= 3:
                renorm_ship(0)
                renorm_ship(1)
            # pair 1: proj windows 2-3 covers AG(0)/AG(1); o_proj laced so
            # every AllGather has ~2 compute blocks of cover before its use
            kv_pair(1)
            for t in range(NQT):
                proj_rope_t(1, t)
            if dbg_phase >= 2:
                attn(2)
            if dbg_phase >= 3:
                renorm_ship(2)
            if dbg_phase >= 4:
                oproj(0)
            if dbg_phase >= 2:
                attn(3)
            if dbg_phase >= 3:
                renorm_ship(3)
            if dbg_phase >= 4:
                oproj(1)
                oproj(2)
                oproj(3)
            else:
                src_t = attnT if dbg_phase >= 2 else qT
                for sb in range(8):
                    os_ = outp.tile([128, EC], FP32, name="osd", tag="out")
                    nc.vector.tensor_copy(os_[:], src_t[:, 0, sb : sb + EC])
                    nc.sync.dma_start(y[sb * 128 : (sb + 1) * 128, :], os_[:])

    return nc


_PROGRAM = [None]


def _get_program():
    if _PROGRAM[0] is None:
        _PROGRAM[0] = build_program()
    return _PROGRAM[0]


def kernel(**inputs) -> np.ndarray:
    nc = _get_program()
    in_maps = host_prepare(**inputs)
    res = run_bass_kernel_spmd(nc, in_maps, list(range(N_CORES)))
    out = np.concatenate([res.results[c]["y"] for c in range(N_CORES)], axis=1)
    return out.reshape(B, S, E)


def kernel_traced(tmpdir=None, **inputs):
    """Like kernel() but with NTFF profiling; returns (out, BassKernelResults)."""
    _install_prof_shim()
    from concourse import bass_utils

    bass_utils.upload_artifacts = lambda d: str(d)
    nc = _get_program()
    in_maps = host_prepare(**inputs)
    res = run_bass_kernel_spmd(
        nc, in_maps, list(range(N_CORES)), trace=True, tmpdir=tmpdir
    )
    out = np.concatenate([res.results[c]["y"] for c in range(N_CORES)], axis=1)
    return out.reshape(B, S, E), res


# revision 24
# speedup vs baseline: 1.1052x; 1.0733x over previous
"""Trainium2 Bass kernel for nn_Attention_38697655337033 (sparse_attention).

GPT-OSS-style sliding-window attention block: QKV proj + YaRN RoPE + GQA
(64 Q heads / 8 KV heads, D=64, window 128, causal) + attention sinks +
o_proj.  Sharded over 8 NeuronCores tensor-parallel by head: core c owns
query heads 8c..8c+7 and KV head c.  o_proj is column-parallel over the
2880 output features (360 per core) after an AllGather of the per-core
attention outputs, chunked by 256-query window so collectives overlap
the next window's compute.

This version is a fully pipelined single-scope schedule:
  - all DRAM tensors are host-swizzled so every DMA is contiguous per
    partition (few, large DMAs; ~350 GB/s per transfer),
  - per-window pipeline: proj(w) -> rope(w) -> attn(w) -> renorm/AG(w),
    with renorm(w-1) broadcast matmuls and oproj(w-1) interleaved so the
    PE never waits on collectives until the tail,
  - rope rotate-half runs on the PE via permutation matmuls (fp32r),
  - scores A/B head-halves run concurrently in disjoint PE row-quadrants,
  - DMA issue is spread across the Sync, Activation and GpSimd queues.

Numerics: projections, scores/PV and o_proj run bf16 operands with fp32
PSUM accumulation; rope runs fp32(r); softmax has no max-subtraction and
folds the sink into the denominator (fp32).

Self-contained: hardcodes all shapes; builds and caches the Bass program
on first call.
"""

import math
import os
import sys
import types

import numpy as np

try:
    import concourse.bass as bass  # noqa: F401
except ImportError:  # pragma: no cover
    sys.path.insert(0, "/opt/trn_rl_repo")

import ml_dtypes

import concourse.bass as bass
import concourse.mybir as mybir
import concourse.tile as tile
from concourse.bass_utils import run_bass_kernel_spmd
from concourse.masks import make_identity
from concourse.tile import ScopedClock

# ---------------------------------------------------------------- constants
B, S, E = 1, 1024, 2880
H, KV, D = 64, 8, 64
WIN = 128
BASE, SCALE, ORIG = 150000.0, 32.0, 4096
BFAST, BSLOW = 32.0, 1.0
SCALING = D ** -0.5  # 0.125, exact power of two -> folded into Wq on host

N_CORES = 8
HL = H // N_CORES          # 8 local query heads
HD_L = HL * D              # 512 local q dims
EC = E // N_CORES          # 360 output columns per core
EP = 2944                  # E padded to 23*128
KT = EP // 128             # 23 contraction tiles for projections
NQT = HL // 2              # 4 head-pair tiles
NQW = S // 256             # 4 query windows of 256
KO = H * D // 128          # 32 o_proj contraction tiles
XW = KT * 256              # 5888 x columns per window

FP32 = mybir.dt.float32
FP32R = mybir.dt.float32r
BF16 = mybir.dt.bfloat16

# ------------------------------------------------------- walrus compat patch
# This container's walrus build rejects instructions with >1 sync-wait
# ("Too many sync wait commands").  Split extra waits onto same-engine NoOp
# carriers, and split the final Tile drain into one drain per wait.
_compat_done = [False]
_carrier_n = [0]


def _install_tile_compat():
    if _compat_done[0]:
        return
    _compat_done[0] = True

    orig_cal = tile.TileContext._commit_and_lower

    def patched_cal(self, inst, original_block, old_bb_map, bb_to_exit_bb):
        if isinstance(inst, mybir.Instruction):
            si = getattr(inst, "sync_info", None)
            if si is not None and len(si.on_wait) > 1:
                waits = list(si.on_wait)
                for w in waits[:-1]:
                    _carrier_n[0] += 1
                    nop = mybir.InstNoOp(
                        name=f"swsplit-{_carrier_n[0]}",
                        engine=inst.engine,
                        sync_info=mybir.SyncInfo(on_wait=[w], on_update=[]),
                        bass_nofuse=True,
                    )
                    self._commit_instruction(nop)
                inst.sync_info = mybir.SyncInfo(
                    on_wait=[waits[-1]], on_update=list(si.on_update)
                )
        return orig_cal(self, inst, original_block, old_bb_map, bb_to_exit_bb)

    tile.TileContext._commit_and_lower = patched_cal

    def patched_dab(self, tick_clock, wait_clock):
        drain_inst = self.nc.sync.drain()
        wait_clock.add_sem_waits(
            drain_inst.ins, ScopedClock({None: tick_clock.global_clock})
        )
        si = drain_inst.ins.sync_info
        if si is not None and len(si.on_wait) > 1:
            waits = list(si.on_wait)
            drain_inst.ins.sync_info = mybir.SyncInfo(on_wait=waits[:1], on_update=[])
            for i in range(1, len(waits)):
                extra = self.nc.sync.drain()
                extra.ins.sync_info = mybir.SyncInfo(
                    on_wait=waits[i : i + 1], on_update=[]
                )
        self.nc.all_engine_barrier()
        assert self.sems is not None
        popped = self.nc._tile_sem_poison_stack.pop()
        assert popped is self._sem_poison
        self.nc.clear_and_free_semaphores(list(self.sems.allocated().values()))
        self.nc.all_engine_barrier()

    tile.TileContext._drain_and_barrier = patched_dab


def _install_ldw_opt():
    """Enable walrus LDWEIGHTS optimization (pull-ahead/FWL) — the stock
    compile command pins it off, which serializes a ~107ns weight load in
    front of every matmul."""
    from concourse import bass_utils

    if getattr(bass_utils, "_ldw_patched", False):
        return
    orig = bass_utils.run_command

    def patched(cmd, *a, **k):
        if isinstance(cmd, list):
            cmd = [
                "--enable-ldw-opt=true" if c == "--enable-ldw-opt=false" else c
                for c in cmd
            ]
        return orig(cmd, *a, **k)

    bass_utils.run_command = patched
    bass_utils._ldw_patched = True


def _install_prof_shim():
    """antenv.axon_hooks is missing in this container; provide it so
    BASS_TRACE-style profiling paths don't crash."""
    try:
        import antenv.axon_hooks  # noqa: F401
        return
    except ImportError:
        pass
    try:
        import antenv
        from trn_agent_boot.trn_boot import _ntff_profile_via_ctypes

        hook = _ntff_profile_via_ctypes("/opt/axon/libaxon_pjrt.so")
    except Exception:
        hook = None
        try:
            import antenv
        except ImportError:
            return
    mod = types.ModuleType("antenv.axon_hooks")
    mod._hook = hook
    mod.get_axon_ntff_profile_hook = lambda: mod._hook

    def _set(h):
        mod._hook = h

    mod.set_axon_ntff_profile_hook = _set
    sys.modules["antenv.axon_hooks"] = mod
    antenv.axon_hooks = mod


# ---------------------------------------------------------------- host prep
def _rope_tables_np(positions):
    """cos/sin YaRN tables, mirroring the reference, in float32."""
    def find_dim(rot):
        return D * math.log(ORIG / (rot * 2 * math.pi)) / (2 * math.log(BASE))

    low = max(find_dim(BFAST), 0.0)
    high = min(find_dim(BSLOW), D // 2 - 1)
    if low == high:
        high += 0.001
    pos_freqs = (BASE ** (np.arange(0, D, 2, dtype=np.float32) / np.float32(D))).astype(
        np.float32
    )
    ramp = np.clip(
        (np.arange(D // 2, dtype=np.float32) - np.float32(low))
        / np.float32(high - low),
        0.0,
        1.0,
    ).astype(np.float32)
    inv_freq = (
        (np.float32(1.0) / (np.float32(SCALE) * pos_freqs)) * ramp
        + (np.float32(1.0) / pos_freqs) * (np.float32(1.0) - ramp)
    ).astype(np.float32)
    mscale = np.float32(0.1 * math.log(SCALE) + 1.0)
    ang = positions.astype(np.float32)[:, None] * inv_freq[None, :]  # [S, 32]
    emb = np.concatenate([ang, ang], axis=-1)  # [S, D]
    return (np.cos(emb) * mscale).astype(np.float32), (np.sin(emb) * mscale).astype(
        np.float32
    )


def _make_masks():
    """Multiplicative [128, 256] masks per slot in the transposed-score
    layout.  Slot s (of 3) covers key block kb = 2Q-1+s for query window Q
    (256 wide).  Entry [j, i2] is 1 when query i2 may attend key j of that
    block:
      slot0: i2 <  j           (keys one block behind the window)
      slot1: j <= i2 <= j+127  (keys in the window's first block)
      slot2: i2 >= j+128       (keys in the window's second block)
    Window 0 simply skips slot 0 (its key block doesn't exist)."""
    j = np.arange(128)[:, None]
    i2 = np.arange(256)[None, :]
    m0 = (i2 < j).astype(np.float32)
    m1 = ((i2 >= j) & (i2 <= j + 127)).astype(np.float32)
    m2 = (i2 >= j + 128).astype(np.float32)
    return m0, m1, m2


def _swap64(m):
    return (m // 64) * 64 + (m % 64 + 32) % 64


def host_prepare(hidden_states, positions, Wq, bq, Wk, bk, Wv, bv, Wo, bo, sinks):
    """Build the 8 per-core input maps (all partition-contiguous layouts)."""
    bf = ml_dtypes.bfloat16
    x = np.asarray(hidden_states, np.float32).reshape(S, E)
    xT = np.zeros((EP, S), np.float32)
    xT[:E] = x.T
    # [p, w, k, s2] flattened to [128, NQW*KT*256]
    x_sw = np.ascontiguousarray(
        xT.reshape(KT, 128, NQW, 256).transpose(1, 2, 0, 3).reshape(128, NQW * XW)
    ).astype(bf)

    cos, sin = _rope_tables_np(np.asarray(positions))
    cosT = np.ascontiguousarray(cos.T)  # [64, S]
    sinT = np.ascontiguousarray(sin.T)
    sgn = np.where(np.arange(D) < D // 2, np.float32(-1.0), np.float32(1.0))
    sinTs = sinT * sgn[:, None]
    cos2 = np.ascontiguousarray(np.concatenate([cosT, cosT], axis=0))  # [128, S]
    sin2s = np.ascontiguousarray(np.concatenate([sinTs, sinTs], axis=0))

    m0, m1, m2 = _make_masks()
    maskAB = np.ascontiguousarray(
        np.concatenate([np.concatenate([m, m], axis=1) for m in (m0, m1, m2)], axis=1)
    ).astype(bf)  # [128, 1536]

    # permutation matrices for rope rotate-half / k duplication (fp32r)
    mm_ = np.arange(128)
    prot = np.zeros((128, 128), np.float32)
    prot[_swap64(mm_), mm_] = 1.0
    pkd = np.zeros((64, 128), np.float32)
    pkd[mm_ % 64, mm_] = 1.0
    pkr = np.zeros((64, 128), np.float32)
    pkr[(mm_ % 64 + 32) % 64, mm_] = 1.0

    # renorm broadcast row-selectors: sel[r, t*128+p] = 1 iff r == 2t + p//64
    sel = np.zeros((8, NQT, 128), np.float32)
    for t in range(NQT):
        sel[2 * t, t, 0:64] = 1.0
        sel[2 * t + 1, t, 64:128] = 1.0
    sel = np.ascontiguousarray(sel.reshape(8, NQT * 128))

    Wq = np.asarray(Wq, np.float32)
    Wk = np.asarray(Wk, np.float32)
    Wv = np.asarray(Wv, np.float32)
    Wo = np.asarray(Wo, np.float32)
    bq = np.asarray(bq, np.float32)
    bk = np.asarray(bk, np.float32)
    bv = np.asarray(bv, np.float32)
    bo = np.asarray(bo, np.float32)
    sinks = np.asarray(sinks, np.float32)

    in_maps = []
    for c in range(N_CORES):
        wq_c = Wq[c * HD_L : (c + 1) * HD_L] * np.float32(SCALING)  # [512, E]
        wqT = np.zeros((EP, HD_L), np.float32)
        wqT[:E] = wq_c.T
        wq_sw = np.ascontiguousarray(
            wqT.reshape(KT, 128, HD_L).transpose(1, 0, 2).reshape(128, KT * HD_L)
        ).astype(bf)

        wkv_c = np.concatenate(
            [Wk[c * D : (c + 1) * D], Wv[c * D : (c + 1) * D]], axis=0
        )  # [128, E]
        wkvT = np.zeros((EP, 128), np.float32)
        wkvT[:E] = wkv_c.T
        wkv_sw = np.ascontiguousarray(
            wkvT.reshape(KT, 128, 128).transpose(1, 0, 2).reshape(128, KT * 128)
        ).astype(bf)

        woT = np.ascontiguousarray(Wo[c * EC : (c + 1) * EC, :].T)  # [4096, 360]
        wo_sw = np.ascontiguousarray(
            woT.reshape(KO, 128, EC).transpose(1, 0, 2).reshape(128, KO * EC)
        ).astype(bf)

        bq_c = (bq[c * HD_L : (c + 1) * HD_L] * np.float32(SCALING)).reshape(4, 128)
        bq_dev = np.ascontiguousarray(bq_c.T)  # [128, 4]
        bkv_dev = np.ascontiguousarray(
            np.concatenate([bk[c * D : (c + 1) * D], bv[c * D : (c + 1) * D]]).reshape(
                128, 1
            )
        )
        bo_dev = np.ascontiguousarray(
            np.broadcast_to(bo[c * EC : (c + 1) * EC].reshape(1, EC), (128, EC))
        )
        esink8 = np.ascontiguousarray(
            np.exp(sinks[c * HL : (c + 1) * HL]).astype(np.float32).reshape(8, 1)
        )

        in_maps.append(
            {
                "x": x_sw,
                "wq": wq_sw,
                "wkv": wkv_sw,
                "wo": wo_sw,
                "bq": bq_dev,
                "bkv": bkv_dev,
                "bo": bo_dev,
                "cos2": cos2,
                "sin2s": sin2s,
                "maskAB": maskAB,
                "esink8": esink8,
                "sel": sel,
                "prot": np.ascontiguousarray(prot),
                "pkd": np.ascontiguousarray(pkd),
                "pkr": np.ascontiguousarray(pkr),
            }
        )
    return in_maps


# ------------------------------------------------------------- device build
def build_program():
    _install_tile_compat()
    _install_prof_shim()
    dbg_no_cc = os.environ.get("DBG_NO_CC") == "1"
    dbg_no_actdma = os.environ.get("DBG_NO_ACTDMA") == "1"
    dbg_no_gpdma = os.environ.get("DBG_NO_GPDMA") == "1"
    dbg_no_memset = os.environ.get("DBG_NO_MEMSET") == "1"
    dbg_phase = int(os.environ.get("DBG_PHASE", "4"))

    nc = bass.Bass("TRN2", target_bir_lowering=False, debug=False, num_devices=N_CORES)

    x_d = nc.declare_dram_parameter("x", [128, NQW * KT * 256], BF16, isOutput=False)
    wq_d = nc.declare_dram_parameter("wq", [128, KT * HD_L], BF16, isOutput=False)
    wkv_d = nc.declare_dram_parameter("wkv", [128, KT * 128], BF16, isOutput=False)
    wo_d = nc.declare_dram_parameter("wo", [128, KO * EC], BF16, isOutput=False)
    bq_d = nc.declare_dram_parameter("bq", [128, 4], FP32, isOutput=False)
    bkv_d = nc.declare_dram_parameter("bkv", [128, 1], FP32, isOutput=False)
    bo_d = nc.declare_dram_parameter("bo", [128, EC], FP32, isOutput=False)
    cos_d = nc.declare_dram_parameter("cos2", [128, S], FP32, isOutput=False)
    sin_d = nc.declare_dram_parameter("sin2s", [128, S], FP32, isOutput=False)
    mask_d = nc.declare_dram_parameter("maskAB", [128, 1536], BF16, isOutput=False)
    esink_d = nc.declare_dram_parameter("esink8", [8, 1], FP32, isOutput=False)
    sel_d = nc.declare_dram_parameter("sel", [8, NQT * 128], FP32R, isOutput=False)
    prot_d = nc.declare_dram_parameter("prot", [128, 128], FP32R, isOutput=False)
    pkd_d = nc.declare_dram_parameter("pkd", [64, 128], FP32R, isOutput=False)
    pkr_d = nc.declare_dram_parameter("pkr", [64, 128], FP32R, isOutput=False)
    y = nc.declare_dram_parameter("y", [S, EC], FP32, isOutput=True)

    Ident = mybir.ActivationFunctionType.Identity
    Exp = mybir.ActivationFunctionType.Exp
    Mult = mybir.AluOpType.mult
    Add = mybir.AluOpType.add

    with tile.TileContext(nc) as tc, nc.allow_low_precision(
        reason="bf16/fp32r operands for PE fast path; accumulation stays fp32"
    ):
        with (
            tc.tile_pool(name="persist", bufs=1) as per,
            tc.tile_pool(name="qb_pool", bufs=3) as qbp,
            tc.tile_pool(name="kvb_pool", bufs=2) as kvbp,
            tc.tile_pool(name="tmp_pool", bufs=3) as tmpp,
            tc.tile_pool(name="pp_pool", bufs=6) as ppp,
            tc.tile_pool(name="d8_pool", bufs=2) as d8p,
            tc.tile_pool(name="r8_pool", bufs=2) as r8p,
            tc.tile_pool(name="at_pool", bufs=8) as atp,
            tc.tile_pool(name="out_pool", bufs=2) as outp,
            tc.tile_pool(name="pj_ps", bufs=2, space="PSUM") as pj,
            tc.tile_pool(name="sc_ps", bufs=4, space="PSUM") as scp,
            tc.tile_pool(name="pv_ps", bufs=2, space="PSUM") as pvp,
            tc.tile_pool(name="dram", bufs=2, space="DRAM") as dram,
        ):
            # ------------------------------------------------ persistent SBUF
            x_all = per.tile([128, NQW, KT, 256], BF16)
            wq_sb = per.tile([128, KT * HD_L], BF16)
            wkv_sb = per.tile([128, KT * 128], BF16)
            wo_sb = per.tile([128, KO * EC], BF16)
            cos_sb = per.tile([128, S], FP32)
            sin_sb = per.tile([128, S], FP32)
            mask_sb = per.tile([128, 1536], BF16)
            qT = per.tile([128, NQT, S], BF16)
            k2T = per.tile([128, S], BF16)
            v_sb = per.tile([128, 8, 66], BF16)
            attnT = per.tile([128, NQT, S], BF16)
            den_seg = per.tile([65, 2048], FP32)
            bq_sb = per.tile([128, 4], FP32)
            bkv_sb = per.tile([128, 1], FP32)
            bo_sb = per.tile([128, EC], FP32)
            esink_sb = per.tile([8, 1], FP32)
            sel_sb = per.tile([8, NQT * 128], FP32R)
            prot_sb = per.tile([128, 128], FP32R)
            pkd_sb = per.tile([64, 128], FP32R)
            pkr_sb = per.tile([64, 128], FP32R)
            ident = per.tile([128, 128], FP32)

            ag_space = {} if dbg_no_cc else {"addr_space": "Shared"}
            ag_out = [
                dram.tile([H * D, 256], BF16, name=f"ag_out{w}", **ag_space)
                for w in range(NQW)
            ]

            # ------------------------------------------- initial loads
            # sync queue: ONLY the big critical-path transfers (wkv + x),
            # windows 0/1 chunk-interleaved since pair-0 proj reads both
            nc.sync.dma_start(wkv_sb[:], wkv_d[:])
            kch = [(0, 6), (6, 12), (12, 18), (18, 23)]
            for k0, k1 in kch:
                nc.sync.dma_start(
                    x_all[:, 0, k0:k1, :], x_d[:, k0 * 256 : k1 * 256]
                )
                nc.sync.dma_start(
                    x_all[:, 1, k0:k1, :], x_d[:, XW + k0 * 256 : XW + k1 * 256]
                )
            for w in range(2, NQW):
                nc.sync.dma_start(
                    x_all[:, w, :, :], x_d[:, w * XW : (w + 1) * XW]
                )
            # scalar(ACT) queue: wq (parallel with sync's wkv/x stream)
            _eng_t = nc.sync if dbg_no_actdma else nc.scalar
            for k0, k1 in kch:
                _eng_t.dma_start(
                    wq_sb[:, k0 * HD_L : k1 * HD_L], wq_d[:, k0 * HD_L : k1 * HD_L]
                )
            # gpsimd queue: full-shape warm collective (absorbs ncfw setup),
            # consts, tables, o_proj weights, second warm collective (absorbs
            # core launch skew before the first real AllGather)
            _eng_g = nc.sync if dbg_no_gpdma else nc.gpsimd
            if not dbg_no_cc:
                warm_in = dram.tile([HD_L, 256], BF16, name="warm_in", tag="warm_in")
                warm_out = dram.tile(
                    [H * D, 256], BF16, name="warm_out", addr_space="Shared"
                )
                warm_out2 = dram.tile(
                    [H * D, 256], BF16, name="warm_out2", addr_space="Shared"
                )
                nc.gpsimd.collective_compute(
                    "AllGather",
                    mybir.AluOpType.bypass,
                    ins=[warm_in[:].opt()],
                    outs=[warm_out[:].opt()],
                    replica_groups=[list(range(N_CORES))],
                )
            _eng_g.dma_start(bq_sb[:], bq_d[:])
            _eng_g.dma_start(bkv_sb[:], bkv_d[:])
            _eng_g.dma_start(esink_sb[:], esink_d[:])
            _eng_g.dma_start(sel_sb[:], sel_d[:])
            _eng_g.dma_start(prot_sb[:], prot_d[:])
            _eng_g.dma_start(pkd_sb[:], pkd_d[:])
            _eng_g.dma_start(pkr_sb[:], pkr_d[:])
            _eng_g.dma_start(cos_sb[:], cos_d[:])
            _eng_g.dma_start(sin_sb[:], sin_d[:])
            _eng_g.dma_start(mask_sb[:], mask_d[:])
            _eng_g.dma_start(bo_sb[:], bo_d[:])
            _eng_g.dma_start(wo_sb[:, : 16 * EC], wo_d[:, : 16 * EC])
            _eng_g.dma_start(wo_sb[:, 16 * EC :], wo_d[:, 16 * EC :])
            if not dbg_no_cc:
                nc.gpsimd.collective_compute(
                    "AllGather",
                    mybir.AluOpType.bypass,
                    ins=[warm_in[:].opt()],
                    outs=[warm_out2[:].opt()],
                    replica_groups=[list(range(N_CORES))],
                )

            make_identity(nc, ident[:])
            if not dbg_no_memset:
                nc.vector.memset(v_sb[:, :, 64:65], 1.0)

            d8_h = {}

            # ------------------------------------------------ window helpers
            def kv_pair(p):
                ps = pj.tile([128, 512], FP32, name="pjkv", tag="pj")
                for k in range(KT):
                    nc.tensor.matmul(
                        ps[:],
                        wkv_sb[:, k * 128 : (k + 1) * 128],
                        x_all[:, 2 * p : 2 * p + 2, k, :],
                        start=k == 0,
                        stop=k == KT - 1,
                    )
                kvb = kvbp.tile([128, 512], FP32R, name="kvb", tag="kvb")
                nc.scalar.activation(kvb[:], ps[:], Ident, bias=bkv_sb[:, 0:1])
                psl = slice(p * 512, (p + 1) * 512)
                kd = pj.tile([128, 512], FP32, name="kd", tag="pj")
                nc.tensor.matmul(
                    kd[:], pkd_sb[:], kvb[0:64, :], start=True, stop=True
                )
                kr = pj.tile([128, 512], FP32, name="kr", tag="pj")
                nc.tensor.matmul(
                    kr[:], pkr_sb[:], kvb[0:64, :], start=True, stop=True
                )
                ktmp = tmpp.tile([128, 512], BF16, name="ktmp", tag="tmp")
                nc.vector.tensor_tensor(ktmp[:], kr[:], sin_sb[:, psl], Mult)
                nc.vector.tensor_tensor(k2T[:, psl], kd[:], cos_sb[:, psl], Mult)
                nc.vector.tensor_tensor(k2T[:, psl], k2T[:, psl], ktmp[:], Add)
                for b in range(4):
                    vt = pj.tile([128, 64], FP32, name="vt", tag="pj")
                    nc.tensor.transpose(
                        vt[:],
                        kvb[64:128, b * 128 : (b + 1) * 128].bitcast(FP32),
                        ident[64:128, 64:128],
                    )
                    nc.vector.tensor_copy(v_sb[:, 4 * p + b, 0:64], vt[:])

            def proj_rope_t(p, t):
                psl = slice(p * 512, (p + 1) * 512)
                ps = pj.tile([128, 512], FP32, name="pjq", tag="pj")
                for k in range(KT):
                    nc.tensor.matmul(
                        ps[:],
                        wq_sb[:, k * HD_L + t * 128 : k * HD_L + (t + 1) * 128],
                        x_all[:, 2 * p : 2 * p + 2, k, :],
                        start=k == 0,
                        stop=k == KT - 1,
                    )
                qb = qbp.tile([128, 512], FP32R, name="qb", tag="qb")
                nc.scalar.activation(qb[:], ps[:], Ident, bias=bq_sb[:, t : t + 1])
                rot = pj.tile([128, 512], FP32, name="rot", tag="pj")
                nc.tensor.matmul(
                    rot[:], prot_sb[:], qb[:], start=True, stop=True
                )
                qTs = qT[:, t, psl]
                qtmp = tmpp.tile([128, 512], BF16, name="qtmp", tag="tmp")
                nc.vector.tensor_tensor(qtmp[:], rot[:], sin_sb[:, psl], Mult)
                nc.vector.tensor_tensor(qTs, qb[:], cos_sb[:, psl], Mult)
                nc.vector.tensor_tensor(qTs, qTs, qtmp[:], Add)

            def attn(w):
                s0 = 1 if w == 0 else 0
                for t in range(NQT):
                    pv = pvp.tile([65, 512], FP32, name="pv", tag="pv")
                    qsl = slice(w * 256, (w + 1) * 256)
                    for slot in range(s0, 3):
                        kb = 2 * w - 1 + slot
                        ksl = slice(kb * 128, (kb + 1) * 128)
                        scA = scp.tile([128, 256], FP32, name="scA", tag="sc")
                        nc.tensor.matmul(
                            scA[:], k2T[0:64, ksl], qT[0:64, t, qsl],
                            start=True, stop=True,
                        )
                        scB = scp.tile([128, 256], FP32, name="scB", tag="sc")
                        nc.tensor.matmul(
                            scB[:], k2T[64:128, ksl], qT[64:128, t, qsl],
                            start=True, stop=True,
                        )
                        p = ppp.tile([128, 512], BF16, name="p", tag="pp")
                        nc.scalar.activation(p[:, 0:256], scA[:], Exp)
                        nc.scalar.activation(p[:, 256:512], scB[:], Exp)
                        nc.vector.tensor_tensor(
                            p[:], p[:], mask_sb[:, slot * 512 : (slot + 1) * 512], Mult
                        )
                        nc.tensor.matmul(
                            pv[:], v_sb[:, kb, 0:65], p[:],
                            start=slot == s0, stop=slot == 2,
                        )
                    nc.vector.tensor_copy(attnT[0:64, t, qsl], pv[0:64, 0:256])
                    nc.vector.tensor_copy(attnT[64:128, t, qsl], pv[0:64, 256:512])
                    nc.scalar.activation(
                        den_seg[64:65, t * 512 : (t + 1) * 512],
                        pv[64:65, 0:512], Ident,
                    )
                d8 = d8p.tile([8, 256], FP32, name="d8", tag="d8")
                (nc.sync if dbg_no_actdma else nc.scalar).dma_start(
                    d8[:], den_seg[64:65, :]
                )
                d8_h[w] = d8

            def renorm_ship(w):
                qsl = slice(w * 256, (w + 1) * 256)
                d8 = d8_h.pop(w)
                nc.vector.tensor_scalar(d8[:], d8[:], esink_sb[:, 0:1], None, Add)
                r8 = r8p.tile([8, 256], FP32R, name="r8", tag="r8")
                nc.vector.reciprocal(r8[:], d8[:])
                for t in range(NQT):
                    bc = scp.tile([128, 256], FP32, name="bc", tag="sc")
                    nc.tensor.matmul(
                        bc[:], sel_sb[:, t * 128 : (t + 1) * 128], r8[:],
                        start=True, stop=True,
                    )
                    nc.vector.tensor_tensor(
                        attnT[:, t, qsl], attnT[:, t, qsl], bc[:], Mult
                    )
                ag_in = dram.tile([HD_L, 256], BF16, name="ag_in", tag="ag_in")
                (nc.sync if dbg_no_gpdma else nc.gpsimd).dma_start(
                    ag_in[:].rearrange("(t p) s -> p t s", p=128),
                    attnT[:, :, qsl],
                )
                if dbg_no_cc:
                    for cc in range(N_CORES):
                        nc.sync.dma_start(
                            ag_out[w][cc * HD_L : (cc + 1) * HD_L, :], ag_in[:]
                        )
                else:
                    nc.gpsimd.collective_compute(
                        "AllGather",
                        mybir.AluOpType.bypass,
                        ins=[ag_in[:].opt()],
                        outs=[ag_out[w][:].opt()],
                        replica_groups=[list(range(N_CORES))],
                    )

            def oproj(w):
                ats = []
                for cch in range(4):
                    at = atp.tile([128, 2048], BF16, name=f"at{cch}", tag="at")
                    eng = nc.sync if (cch % 2 == 0 or dbg_no_actdma) else nc.scalar
                    eng.dma_start(
                        at[:],
                        ag_out[w][cch * 1024 : (cch + 1) * 1024, :].rearrange(
                            "(k p) s -> p k s", p=128
                        ),
                    )
                    ats.append(at)
                for i in range(2):
                    po = pvp.tile([128, EC], FP32, name="po", tag="pv")
                    for k in range(KO):
                        at = ats[k // 8]
                        kk = k % 8
                        nc.tensor.matmul(
                            po[:],
                            at[:, kk * 256 + i * 128 : kk * 256 + i * 128 + 128],
                            wo_sb[:, k * EC : (k + 1) * EC],
                            start=k == 0,
                            stop=k == KO - 1,
                        )
                    os_ = outp.tile([128, EC], FP32, name="os", tag="out")
                    nc.vector.tensor_tensor(os_[:], po[:], bo_sb[:], Add)
                    (nc.sync if dbg_no_gpdma else nc.gpsimd).dma_start(
                        y[(2 * w + i) * 128 : (2 * w + i + 1) * 128, :], os_[:]
                    )

            # ------------------------------------------------ the pipeline
            # pair 0: proj windows 0-1, their attention + shipping
            kv_pair(0)
            for t in range(NQT):
                proj_rope_t(0, t)
            if dbg_phase >= 2:
                attn(0)
                attn(1)
            if dbg_phase >= 3:
                renorm_ship(0)
                renorm_ship(1)
            # pair 1: proj windows 2-3 covers AG(0)/AG(1); o_proj laced so
            # every AllGather has ~2 compute blocks of cover before its use
            kv_pair(1)
            for t in range(NQT):
                proj_rope_t(1, t)
            if dbg_phase >= 2:
                attn(2)
            if dbg_phase >= 3:
                renorm_ship(2)
            if dbg_phase >= 4:
                oproj(0)
            if dbg_phase >= 2:
                attn(3)
            if dbg_phase >= 3:
                renorm_ship(3)
            if dbg_phase >= 4:
                oproj(1)
                oproj(2)
                oproj(3)
            else:
                src_t = attnT if dbg_phase >= 2 else qT
                for sb in range(8):
                    os_ = outp.tile([128, EC], FP32, name="osd", tag="out")
                    nc.vector.tensor_copy(os_[:], src_t[:, 0, sb : sb + EC])
                    nc.sync.dma_start(y[sb * 128 : (sb + 1) * 128, :], os_[:])

    return nc


_PROGRAM = [None]


def _get_program():
    if _PROGRAM[0] is None:
        _PROGRAM[0] = build_program()
    return _PROGRAM[0]


def kernel(**inputs) -> np.ndarray:
    nc = _get_program()
    in_maps = host_prepare(**inputs)
    res = run_bass_kernel_spmd(nc, in_maps, list(range(N_CORES)))
    out = np.concatenate([res.results[c]["y"] for c in range(N_CORES)], axis=1)
    return out.reshape(B, S, E)


def kernel_traced(tmpdir=None, **inputs):
    """Like kernel() but with NTFF profiling; returns (out, BassKernelResults)."""
    _install_prof_shim()
    from concourse import bass_utils

    bass_utils.upload_artifacts = lambda d: str(d)
    nc = _get_program()
    in_maps = host_prepare(**inputs)
    res = run_bass_kernel_spmd(
        nc, in_maps, list(range(N_CORES)), trace=True, tmpdir=tmpdir
    )
    out = np.concatenate([res.results[c]["y"] for c in range(N_CORES)], axis=1)
    return out.reshape(B, S, E), res
